# revision 18
# baseline (speedup 1.0000x reference)
"""Trainium2 Bass kernel for nn_AttentionToVec (B=8, N=4096, E=1024, H=16, D=64).

Strategy: pure data-parallel over batch (1 batch element per NeuronCore), NO
collectives.  Each core computes its own row's full MLP with the complete
W1/W2 (profiling showed the AllGather/ReduceScatter + cc-barrier of the
tensor-parallel MLP cost ~100us, far more than the extra weight traffic).

x is read from HBM exactly ONCE (fp16, 8.4MB): phase A transposes each
[128,128] block on the tensor engine (PSUM) and copies it back to SBUF
(vector engine), then computes att logits per n-tile with the transposed
block as the stationary operand -- so attention runs as a single fused
pass per tile: transpose -> copy -> att-matmul -> exp -> y/z-matmul.

DMA discipline: all loads go through the single HWDGE sync queue, which
drains FIFO in issue order; every stream tensor is fully resident in SBUF
(no pool-buffer gating), so issue order IS arrival order:
  x (fp16, 8.4MB) -> Wv (fp16, 2.1MB) -> W1 (fp8, 4.2MB) -> W2 (fp16,
  8.4MB, sliced by q-groups so the W2 matmuls pipeline behind arrival).

Dtypes (validated vs reference on host, rel-err ~8.9e-3 vs 2e-2 budget):
  W1 fp8e4m3 (mixed with fp16 moving s), everything else fp16; all matmul
  accumulation fp32 in PSUM.

Algebra (host does weight-only folding):
  - att logits = x @ w_att,  w_att[e,h] = sum_d W_k[e, h*D+d] * query[h,d]
    (the k-projection bias cancels inside softmax over n).
  - y[h,:] = sum_n exp_att[n,h] * x[n,:]  (deferred 1/Z normalization)
  - sampled[e] = (y[h(e),:] @ W_v[:, e]) + b_v[e],  h(e)=e//D.  Phase C
    computes ONLY the needed diagonal blocks, directly transposed:
    sfT_j[m, i] = sf[2j+i, 128j+m] so s[128j+m] = sfT_j[m, m//64].
  - MLP per-core on its own row, hidden laid out as zT[p, q] = z[128q+p]
    so gelu runs across all 128 partitions.
"""

import numpy as np

B = 8
N = 4096
E = 1024
H = 16
D = 64
HID = 4096
NCORES = 8

_CACHE = {}


def _build():
    import concourse.bacc as bacc
    import concourse.mybir as mybir
    from concourse import tile

    f32 = mybir.dt.float32
    f16 = mybir.dt.float16
    f8 = mybir.dt.float8e4
    Act = mybir.ActivationFunctionType
    Alu = mybir.AluOpType

    nc = bacc.Bacc(None, target_bir_lowering=False, debug=True, num_devices=NCORES)

    x16 = nc.dram_tensor("x16", [N, E], f16, kind="ExternalInput")
    # wpack cols 0:128 = watt chunks (col 16c+h = w_att[128c+p, h]),
    #       cols 128:256 = 128x128 identity
    wpack = nc.dram_tensor("wpack", [128, 256], f16, kind="ExternalInput")
    # packed [128, 80] f32: cols 0:32 maskn, 32:40 bvT, 40:72 b1T, 72:80 b2T
    cpack = nc.dram_tensor("cpack", [128, 80], f32, kind="ExternalInput")
    wv = nc.dram_tensor("wv", [E, E], f16, kind="ExternalInput")
    w1 = nc.dram_tensor("w1", [E, HID], f8, kind="ExternalInput")
    w2 = nc.dram_tensor("w2", [HID, E], f16, kind="ExternalInput")
    # outT[p, j] = out_row[128*j + p]; host reassembles
    out = nc.dram_tensor("out", [128, 8], f32, kind="ExternalOutput")

    with tile.TileContext(nc) as tc:
        with (
            tc.tile_pool(name="consts", bufs=1) as consts,
            tc.tile_pool(name="xp", bufs=1) as xp,
            tc.tile_pool(name="xblk", bufs=2) as xblk,
            tc.tile_pool(name="wvp", bufs=1) as wvp,
            tc.tile_pool(name="w1p", bufs=1) as w1p,
            tc.tile_pool(name="w2p", bufs=1) as w2p,
            tc.tile_pool(name="work", bufs=1) as work,
        ):
            ones_s = consts.tile([128, 1], f16)
            nc.vector.memset(ones_s[:], 1.0)

            # ---- all DMA triggers in FIFO priority order ----
            x_s = xp.tile([128, 32, E], f16)
            xr = x16.ap().rearrange("(g r p) e -> g p r e", g=8, p=128)
            nc.sync.dma_start(out=x_s[:, 0:4, :], in_=xr[0])

            wp_s = consts.tile([128, 256], f16)
            nc.sync.dma_start(out=wp_s[:], in_=wpack[:, :])
            cp_s = consts.tile([128, 80], f32)
            nc.sync.dma_start(out=cp_s[:], in_=cpack[:, :])

            for g in range(1, 8):
                nc.sync.dma_start(out=x_s[:, 4 * g : 4 * (g + 1), :], in_=xr[g])

            wv_s = wvp.tile([128, 8, 8, 128], f16)
            nc.sync.dma_start(
                out=wv_s[:],
                in_=wv.ap().rearrange("(c p) (j m) -> p c j m", p=128, m=128),
            )
            w1_s = w1p.tile([128, 8, 32, 128], f8)
            w1r = w1.ap().rearrange("(c p) (g q m) -> g p c q m", p=128, g=2, m=128)
            for g in range(2):
                nc.sync.dma_start(out=w1_s[:, :, 16 * g : 16 * (g + 1), :], in_=w1r[g])
            w2_s = w2p.tile([128, 32, 8, 128], f16)
            w2r = w2.ap().rearrange("(g q p) (r m) -> g p q r m", g=4, p=128, m=128)
            for g in range(4):
                nc.sync.dma_start(out=w2_s[:, 8 * g : 8 * (g + 1), :, :], in_=w2r[g])

            identity128 = wp_s[:, 128:256]
            identity16 = wp_s[0:16, 128:144]

            # ---- Phase A+B fused, single pass over x ----
            psB_cm = tc.tile_pool(name="psB", bufs=1, space="PSUM")
            psB = psB_cm.__enter__()
            psAt_cm = tc.tile_pool(name="psAt", bufs=2, space="PSUM")
            psAt = psAt_cm.__enter__()
            psTx_cm = tc.tile_pool(name="psTx", bufs=2, space="PSUM")
            psTx = psTx_cm.__enter__()

            att_n = work.tile([128, 32 * H], f16)
            y_ps = psB.tile([H, E], f32, tag="acc")
            z_ps = psB.tile([H, 1], f32, tag="accz")
            for t in range(32):
                # transpose the 8 [128,128] blocks of x tile t: PSUM, then SBUF
                xt_ps = psTx.tile([128, 8, 128], f16, tag="tx")
                for c in range(8):
                    nc.tensor.transpose(
                        xt_ps[:, c, :],
                        x_s[:, t, 128 * c : 128 * (c + 1)],
                        identity128,
                    )
                xb = xblk.tile([128, 8, 128], f16, tag="xb")
                for c in range(8):
                    nc.vector.tensor_copy(xb[:, c, :], xt_ps[:, c, :])
                # att_t[n, h] = sum_e x[n, e] w_att[e, h]
                att_t = psAt.tile([128, H], f32, tag="at")
                for c in range(8):
                    nc.tensor.matmul(
                        att_t[:],
                        xb[:, c, :],
                        wp_s[:, 16 * c : 16 * (c + 1)],
                        start=(c == 0),
                        stop=(c == 7),
                    )
                # p = exp(att + mask)
                nc.scalar.activation(
                    att_n[:, H * t : H * (t + 1)],
                    att_t[:],
                    Act.Exp,
                    bias=cp_s[:, t : t + 1],
                )
                # y += p.T @ x, z += p.T @ 1
                lhs = att_n[:, H * t : H * (t + 1)]
                nc.tensor.matmul(
                    y_ps[:, 0:512],
                    lhs,
                    x_s[:, t, 0:512],
                    start=(t == 0),
                    stop=(t == 31),
                )
                nc.tensor.matmul(
                    y_ps[:, 512:1024],
                    lhs,
                    x_s[:, t, 512:1024],
                    start=(t == 0),
                    stop=(t == 31),
                )
                nc.tensor.matmul(
                    z_ps[:],
                    lhs,
                    ones_s[:],
                    start=(t == 0),
                    stop=(t == 31),
                )
            psTx_cm.__exit__(None, None, None)
            psAt_cm.__exit__(None, None, None)

            # normalize: y = y / z
            rz = work.tile([H, 1], f32)
            nc.vector.reciprocal(rz[:], z_ps[:, 0:1])
            y_s = work.tile([H, E], f16)
            nc.vector.tensor_scalar_mul(y_s[:], y_ps[:], rz[:])
            psB_cm.__exit__(None, None, None)

            # yT[e, h] chunks (fp16) for phase C
            psTr_cm = tc.tile_pool(name="psTr", bufs=2, space="PSUM")
            psTr = psTr_cm.__enter__()
            yT = work.tile([128, 8 * H], f16)
            for j in range(8):
                tr2 = psTr.tile([128, H], f16, tag="tr")
                nc.tensor.transpose(
                    tr2[:], y_s[:, 128 * j : 128 * (j + 1)], identity16
                )
                nc.vector.tensor_copy(yT[:, H * j : H * (j + 1)], tr2[:])
            psTr_cm.__exit__(None, None, None)

            # ---- Phase C: diagonal blocks of sf = y @ Wv, directly transposed.
            # sfT_j[m, i] = sf[2j+i, 128j+m]; s[128j+m] = sfT_j[m, m//64].
            psC_cm = tc.tile_pool(name="psC", bufs=2, space="PSUM")
            psC = psC_cm.__enter__()
            s_f = work.tile([128, 8], f32)
            for j in range(8):
                sfT = psC.tile([128, 2], f32, tag="sf")
                for c in range(8):
                    nc.tensor.matmul(
                        sfT[:],
                        wv_s[:, c, j, :],
                        yT[:, 16 * c + 2 * j : 16 * c + 2 * j + 2],
                        start=(c == 0),
                        stop=(c == 7),
                    )
                nc.vector.tensor_copy(s_f[0:64, j : j + 1], sfT[0:64, 0:1])
                nc.vector.tensor_copy(s_f[64:128, j : j + 1], sfT[64:128, 1:2])
            psC_cm.__exit__(None, None, None)

            nc.vector.tensor_add(s_f[:], s_f[:], cp_s[:, 32:40])
            s16 = work.tile([128, 8], f16)
            nc.vector.tensor_copy(s16[:], s_f[:])

            # ---- Phase E: full MLP for this core's row ----
            psM_cm = tc.tile_pool(name="psM", bufs=1, space="PSUM")
            psM = psM_cm.__enter__()
            zT_ps = psM.tile([128, 32], f32, tag="z")
            for q in range(32):
                for c in range(8):
                    nc.tensor.matmul(
                        zT_ps[:, q : q + 1],
                        w1_s[:, c, q, :],
                        s16[:, c : c + 1],
                        start=(c == 0),
                        stop=(c == 7),
                    )

            # gelu (tanh approx, matches jax.nn.gelu default)
            z_s = work.tile([128, 32], f32, tag="zs")
            nc.vector.tensor_add(z_s[:], zT_ps[:], cp_s[:, 40:72])
            sq = work.tile([128, 32], f32, tag="ga")
            nc.scalar.activation(sq[:], z_s[:], Act.Square)
            cube = work.tile([128, 32], f32, tag="gb")
            nc.vector.tensor_mul(cube[:], sq[:], z_s[:])
            uu = work.tile([128, 32], f32, tag="ga")
            nc.vector.scalar_tensor_tensor(
                uu[:], cube[:], 0.044715, z_s[:], Alu.mult, Alu.add
            )
            th = work.tile([128, 32], f32, tag="gb")
            nc.scalar.activation(th[:], uu[:], Act.Tanh, scale=0.7978845608028654)
            hh = work.tile([128, 32], f32, tag="ga")
            nc.vector.scalar_tensor_tensor(
                hh[:], th[:], 1.0, z_s[:], Alu.add, Alu.mult
            )
            h16 = work.tile([128, 32], f16, tag="h16")
            nc.vector.tensor_scalar_mul(h16[:], hh[:], 0.5)

            # oT chains partial-accumulate per q-group so they pipeline
            # behind the 4 sliced w2 DMAs.  A start flag marks the whole 2KB
            # zero-region pending-zero, so only the very first matmul of the
            # tile may carry it; later first-touches of other columns still
            # overwrite via the lazy pending-zero bytes.
            oT_ps = psM.tile([128, 8], f32, tag="o")
            for g in range(4):
                for r in range(8):
                    for q in range(8 * g, 8 * (g + 1)):
                        nc.tensor.matmul(
                            oT_ps[:, r : r + 1],
                            w2_s[:, q, r, :],
                            h16[:, q : q + 1],
                            start=(g == 0 and r == 0 and q == 0),
                            stop=(g == 3 and r == 7 and q == 31),
                            skip_group_check=True,
                        )

            of = work.tile([128, 8], f32, tag="of")
            nc.vector.tensor_add(of[:], oT_ps[:], cp_s[:, 72:80])
            nc.vector.tensor_add(of[:], of[:], s_f[:])
            nc.sync.dma_start(out=out[:, :], in_=of[:])
            psM_cm.__exit__(None, None, None)

    return nc


def get_nc():
    if "nc" not in _CACHE:
        nc = _build()
        nc.finalize()
        _CACHE["nc"] = nc
    return _CACHE["nc"]


def build_in_maps(x, mask, W_kv, b_kv, query, W1, b1, W2, b2):
    """Host-side shard prep. Weight-only algebra + layout transforms."""
    import ml_dtypes

    f16 = np.dtype(np.float16)
    f8 = np.dtype(ml_dtypes.float8_e4m3)

    x = np.asarray(x, np.float32)
    mask = np.asarray(mask)
    W_kv = np.asarray(W_kv, np.float32)
    b_kv = np.asarray(b_kv, np.float32)
    query = np.asarray(query, np.float32)
    W1 = np.asarray(W1, np.float32)
    b1 = np.asarray(b1, np.float32)
    W2 = np.asarray(W2, np.float32)
    b2 = np.asarray(b2, np.float32)

    W_k = W_kv[:, :E]
    W_v = W_kv[:, E:]
    # fold the per-head query into the k-projection: [E, H]
    w_att = np.einsum("ehd,hd->eh", W_k.reshape(E, H, D), query).astype(np.float32)

    addmask = np.where(mask[:, :, 0], np.float32(-1e30), np.float32(0.0))  # [B, N]

    wv_c = np.ascontiguousarray(W_v.astype(f16))
    w1_c = np.ascontiguousarray(W1.astype(f8))
    w2_c = np.ascontiguousarray(W2.astype(f16))

    wpack = np.zeros((128, 256), np.float32)
    # wpack[p, 16c+h] = w_att[128c+p, h]
    wpack[:, 0:128] = w_att.reshape(8, 128, H).transpose(1, 0, 2).reshape(128, 128)
    wpack[:, 128:256] = np.eye(128, dtype=np.float32)
    wpack_c = np.ascontiguousarray(wpack.astype(f16))

    cpack_base = np.zeros((128, 80), np.float32)
    cpack_base[:, 32:40] = b_kv[E:].reshape(8, 128).T
    cpack_base[:, 40:72] = b1.reshape(32, 128).T
    cpack_base[:, 72:80] = b2.reshape(8, 128).T

    in_maps = []
    for c in range(NCORES):
        cp = cpack_base.copy()
        # maskn[p, t] = addmask[n = 128*t + p]
        cp[:, 0:32] = addmask[c].reshape(32, 128).T
        in_maps.append(
            {
                "x16": np.ascontiguousarray(x[c].astype(f16)),
                "wpack": wpack_c,
                "cpack": cp,
                "wv": wv_c,
                "w1": w1_c,
                "w2": w2_c,
            }
        )
    return in_maps


def kernel(**inputs):
    from concourse.bass_utils import run_bass_kernel_spmd

    in_maps = build_in_maps(**inputs)
    nc = get_nc()
    res = run_bass_kernel_spmd(nc, in_maps, list(range(NCORES)), trace=False)
    # out is [128, 8] with out_row[128*j + p] = out[p, j]
    return np.stack(
        [np.asarray(res.results[c]["out"]).T.reshape(-1) for c in range(NCORES)]
    ).astype(np.float32)
